# revision 54
# baseline (speedup 1.0000x reference)
"""AWQ 4-bit quantized linear layer on 8 Trainium2 NeuronCores.

Computes out = x @ W.T + bias where W[o,i] = (q[o,i] - z[o,i//128]) * s[o,i//128],
q/z packed 8x int4 per int32.

Sharding: column-parallel (tensor-parallel on out_features). Each of the 8
cores gets the weight rows [c*512, (c+1)*512) and the full activation
(shipped pre-transposed in bf16).

Layout: qweight is host-repacked (pure nibble shuffle) into [in_features,
out_words] order so the on-chip unpack lands directly in the matmul's
[i-partition, j, k, o] layout -- no PE transposes and no PSUM->SBUF copies.
The k-tile size equals the AWQ group size (128), so each k-tile sees a
single scale column; scale and zero*scale ship pre-broadcast across the 128
partitions in chunk-major order (contiguous 1MB loads on the otherwise-idle
SWDGE queue). Dequant is 3 dense DVE ops per nibble plane (shift+mask, *s,
-z*s), software-pipelined in j-pairs; W is double-buffered and written
j-major so every DVE op is dense. The matmul stream (2048 MMs, 512-wide
moving operand, same-bank PSUM accumulation) runs un-interleaved at the
measured ~207-216 ns/MM roofline; x tiles stream on the two alternating
HWDGE rings and output stores get the SWDGE queue to avoid head-of-line
blocking a load behind a store whose data isn't ready.
"""

import os
import sys

for _p in ("/opt/trn_rl_repo", "/root/.axon_site/_ro/trn_rl_repo"):
    if os.path.isdir(_p) and _p not in sys.path:
        sys.path.insert(0, _p)

import numpy as np
import ml_dtypes

import concourse.bass as bass
import concourse.tile as tile
from concourse import bacc, mybir

# Full-problem shapes (hardcoded; harness contract)
B_FULL = 8192
I_FULL = 4096
O_FULL = 4096
N_CORES = 8
GROUP = 128
PACK = 8

BF16 = mybir.dt.bfloat16
F32 = mybir.dt.float32
I32 = mybir.dt.int32


def build_bass(B, I, OS, m_super=512, repeat=1,
               do_dequant=True, x_per_ms=True, do_stores=True, n_split=1,
               do_mm=True, dq_loads=True, wt_jmajor=True, store_q="gpsimd",
               interleave=False, x_ring="alt"):
    """Build the per-core SPMD program.

    B: batch rows, I: in_features, OS: out_features per core.
    m_super: batch columns processed per super-block (multiple of 128).
    repeat: run the whole body N times (hardware For_i loop) - used only
    for timing measurements (wall-clock slope vs repeat).
    do_dequant/x_per_ms/do_stores/n_split: ablation knobs for perf
    microbenchmarks (defaults = the real kernel).
    """
    KT = I // 128          # k-tiles (contraction); one AWQ group per k-tile
    NG = I // GROUP        # quantization groups == KT
    NW = OS // PACK        # packed words per i-row (o-direction packing)
    MSn = B // m_super     # m super-blocks
    M4 = m_super // 128    # 128-row m-tiles per super-block
    KCH = 8                # k-tiles dequantized per chunk
    NKC = KT // KCH

    nc = bacc.Bacc("TRN2", target_bir_lowering=False)

    xT_d = nc.dram_tensor("xT", [I, B], BF16, kind="ExternalInput")
    qw_d = nc.dram_tensor("qw", [I, NW], I32, kind="ExternalInput")
    # scale / zero*scale pre-broadcast across partitions, chunk-major so
    # each k-chunk is one contiguous HWDGE load
    sj_d = nc.dram_tensor("sj", [NKC, 128, PACK, KCH, NW], BF16,
                          kind="ExternalInput")
    zj_d = nc.dram_tensor("zj", [NKC, 128, PACK, KCH, NW], BF16,
                          kind="ExternalInput")
    bi_d = nc.dram_tensor("bi", [OS], F32, kind="ExternalInput")
    # bf16 stores halve the store traffic (host upcasts; ~0.1% rms)
    out_d = nc.dram_tensor("out", [B, OS], BF16, kind="ExternalOutput")

    with tile.TileContext(nc) as tc:
        with (
            tc.tile_pool(name="const", bufs=1) as const,
            tc.tile_pool(name="wt", bufs=2) as wtp,
            tc.tile_pool(name="dq", bufs=2) as dq,
            tc.tile_pool(name="xp", bufs=2) as xp,
            tc.tile_pool(name="ob", bufs=2) as ob,
            tc.tile_pool(name="ps", bufs=8, space="PSUM") as ps,
        ):
            rep_ctx = tc.For_i(0, repeat, 1) if repeat > 1 else None
            if rep_ctx is not None:
                rep_ctx.__enter__()

            # bias broadcast to [128, OS] (varies along free dim of out tiles)
            bias_bc = const.tile([128, OS], F32)
            nc.gpsimd.dma_start(
                out=bias_bc[:],
                in_=bass.AP(tensor=bi_d[:].tensor, offset=0,
                            ap=[[0, 128], [1, OS]]),
            )

            # Dequantized weight bf16, double-buffered so the next
            # repeat-iteration's dequant does not WAR-stall on this
            # iteration's trailing matmuls. j-major layout keeps every DVE
            # dequant write dense (2x mode); the matmul moving AP handles
            # the [j, w] stride.
            if wt_jmajor:
                WT = wtp.tile([128, PACK, KT, NW], BF16, name="WT", tag="WT")

                def wt_mov(k, sp):
                    assert n_split == 1
                    return WT[:, :, k, :]

                def wt_dq(j, ksl):
                    return WT[:, j, ksl, :]
            else:
                WT = wtp.tile([128, KT, OS], BF16, name="WT", tag="WT")

                def wt_mov(k, sp):
                    return WT[:, k, sp * NSP:(sp + 1) * NSP]

                def wt_dq(j, ksl):
                    return WT[:, ksl, 64 * j:64 * j + 64]

            # packed weight, [i-partition, kt, word] (word w holds o = 64j+w)
            qw_sb = dq.tile([128, KT, NW], I32, name="qw_sb", tag="qw_sb",
                            bufs=2)
            qw_v = qw_d.rearrange("(kt p) w -> p kt w", p=128)
            nc.sync.dma_start(qw_sb[:], qw_v)

            # xT viewed as [p, kt, b] so one DMA loads all k-tiles of a
            # super-block (amortizes HWDGE fixed cost)
            xT_v = xT_d.rearrange("(kt p) b -> p kt b", p=128)
            out_v = out_d.rearrange("(ms m4 p) o -> ms p m4 o", p=128, m4=M4)

            def load_x(ms):
                xtile = xp.tile([128, KT, m_super], BF16, name="xtile",
                                tag="xtile")
                if x_ring == "alt":
                    eng = nc.sync if ms % 2 == 0 else nc.scalar
                else:
                    eng = getattr(nc, x_ring)
                eng.dma_start(
                    xtile[:], xT_v[:, :, ms * m_super:(ms + 1) * m_super]
                )
                return xtile

            NSP = OS // n_split

            def mm_run(pss, xtile, m4, ks):
                # consecutive matmuls into the SAME psum bank (avoids
                # per-instruction psum bank cycling)
                for sp in range(n_split):
                    for k in ks:
                        nc.tensor.matmul(
                            pss[m4 * n_split + sp][:],
                            xtile[:, k, m4 * 128:(m4 + 1) * 128],
                            wt_mov(k, sp),
                            start=(k == 0),
                            stop=(k == KT - 1),
                        )

            def evict(pss, ms, force_store=False):
                o_sb = ob.tile([128, M4, OS], BF16, name="o_sb", tag="o_sb",
                               bufs=4)
                for m4 in range(M4):
                    for sp in range(n_split):
                        nc.vector.tensor_add(
                            o_sb[:, m4, sp * NSP:(sp + 1) * NSP],
                            pss[m4 * n_split + sp][:],
                            bias_bc[:, sp * NSP:(sp + 1) * NSP])
                if do_stores or force_store:
                    # stores on their own queue: sharing a FIFO queue with
                    # any load means a store (whose data isn't ready until
                    # the super-block finishes) head-of-line blocks it
                    getattr(nc, store_q).dma_start(out_v[ms], o_sb[:])

            # ---- dequant interleaved with the first two super-blocks ----
            n_inter = 2 if n_split == 1 else 1   # super-blocks in flight
            if do_mm and interleave:
                xtile0 = load_x(0)
                xtile1 = (load_x(1) if x_per_ms else xtile0) if n_inter == 2 \
                    else xtile0
                pss01 = [
                    [ps.tile([128, NSP], F32, name="acc", tag="acc")
                     for _ in range(M4 * n_split)]
                    for _ in range(n_inter)
                ]
            for kc in range(NKC):
                ksl = slice(kc * KCH, (kc + 1) * KCH)
                if do_dequant:
                    # this chunk's pre-broadcast scale / zero*scale
                    # (contiguous 2MB loads on the Activation ring)
                    s_bc = dq.tile([128, PACK, KCH, NW], BF16, name="s_bc",
                                   tag="s_bc", bufs=2)
                    zs_bc = dq.tile([128, PACK, KCH, NW], BF16, name="zs_bc",
                                    tag="zs_bc", bufs=2)
                    if dq_loads:
                        # SWDGE queue: contiguous 1MB loads on the queue
                        # that is otherwise idle until the stores begin, so
                        # they neither delay the x loads on the HWDGE rings
                        # nor get head-of-line blocked themselves
                        nc.gpsimd.dma_start(s_bc[:], sj_d[kc])
                        nc.gpsimd.dma_start(zs_bc[:], zj_d[kc])
                    else:
                        nc.gpsimd.memset(s_bc[:], 0.01)
                        nc.gpsimd.memset(zs_bc[:], 0.0)

                    # software-pipelined in j-pairs so each DVE op's input
                    # was produced two instructions earlier (hides the
                    # SBUF read-write bubble between dependent ops)
                    def emit_ts(j):
                        # nib = (word >> 4j) & 0xF (bitVec ops cannot cast;
                        # the mult converts int32 inputs before the ALU)
                        nib = dq.tile([128, KCH, NW], I32, name="nib",
                                      tag="nib")
                        nc.vector.tensor_scalar(
                            out=nib[:],
                            in0=qw_sb[:, ksl, :],
                            scalar1=4 * j,
                            scalar2=0xF,
                            op0=mybir.AluOpType.logical_shift_right,
                            op1=mybir.AluOpType.bitwise_and,
                        )
                        return nib

                    def emit_mul(j, nib):
                        nibf = dq.tile([128, KCH, NW], BF16, name="nibf",
                                       tag="nibf")
                        nc.vector.tensor_tensor(
                            out=nibf[:], in0=nib[:], in1=s_bc[:, j],
                            op=mybir.AluOpType.mult)
                        return nibf

                    def emit_sub(j, nibf):
                        # W[i, k, o=64j+w] = nib*s - z*s
                        nc.vector.tensor_tensor(
                            out=wt_dq(j, ksl),
                            in0=nibf[:], in1=zs_bc[:, j],
                            op=mybir.AluOpType.subtract)

                    for jp in range(0, PACK, 2):
                        n0 = emit_ts(jp)
                        n1 = emit_ts(jp + 1)
                        f0 = emit_mul(jp, n0)
                        f1 = emit_mul(jp + 1, n1)
                        emit_sub(jp, f0)
                        emit_sub(jp + 1, f1)
                else:
                    # timing ablation: fill the chunk with finite garbage
                    if wt_jmajor:
                        nc.vector.memset(WT[:, :, ksl, :], 0.5)
                    else:
                        nc.vector.memset(WT[:, ksl, :], 0.5)
                # this chunk's k-tiles for super-blocks 0 and 1 (keeps PE
                # fed while the next chunk dequantizes)
                if do_mm and interleave:
                    ks = list(range(kc * KCH, (kc + 1) * KCH))
                    for msi in range(n_inter):
                        for m4 in range(M4):
                            mm_run(pss01[msi], (xtile0, xtile1)[msi], m4, ks)
            if not do_mm:
                # diagnostic builds: tiny consumer so the dequant chain
                # cannot be elided
                pd = ps.tile([128, NSP], F32, name="acc", tag="acc")
                stat = WT[:, 0, 0:2, :] if wt_jmajor else WT[:, 0, 0:128]
                nc.tensor.matmul(pd[:], stat, wt_mov(KT - 1, 0),
                                 start=True, stop=True)
                o_sb = ob.tile([128, M4, OS], F32, name="o_sb", tag="o_sb")
                nc.vector.tensor_add(o_sb[:, 0, :NSP], pd[:],
                                     bias_bc[:, :NSP])
                nc.scalar.dma_start(out_v[0], o_sb[:])
            if do_mm:
                if interleave:
                    for msi in range(n_inter):
                        evict(pss01[msi], msi)
                    ms_start = n_inter
                else:
                    ms_start = 0
                    xtile0 = None

                # ---- remaining super-blocks ----
                for ms in range(ms_start, MSn):
                    if x_per_ms or xtile0 is None:
                        xtile = load_x(ms)
                        if xtile0 is None:
                            xtile0 = xtile
                    else:
                        xtile = xtile0
                    pss = [ps.tile([128, NSP], F32, name="acc", tag="acc")
                           for _ in range(M4 * n_split)]
                    for m4 in range(M4):
                        mm_run(pss, xtile, m4, range(KT))
                    evict(pss, ms, force_store=(ms == MSn - 1))

            if rep_ctx is not None:
                rep_ctx.__exit__(None, None, None)

    nc.compile()
    return nc


_NC_CACHE = {}


def _get_nc(B, I, OS, repeat=1):
    key = (B, I, OS, repeat)
    if key not in _NC_CACHE:
        _NC_CACHE[key] = build_bass(B, I, OS, repeat=repeat)
    return _NC_CACHE[key]


def _unpack_int4_np(packed):
    """[N, W] int32 -> [N, W*8] uint8 nibbles (low nibble first)."""
    u = packed.view(np.uint32)
    shifts = (np.arange(PACK, dtype=np.uint32) * 4)[None, None, :]
    vals = (u[:, :, None] >> shifts) & np.uint32(0xF)
    return vals.reshape(packed.shape[0], -1).astype(np.uint8)


def make_in_maps(x, qweight, qzeros, scales, bias, n_cores=N_CORES):
    O = qweight.shape[0]
    I = x.shape[1]
    OS = O // n_cores
    NW = OS // PACK
    NG = I // GROUP
    xT = np.ascontiguousarray(x.T).astype(ml_dtypes.bfloat16)
    q4 = _unpack_int4_np(qweight)                  # [O, I]
    z4 = _unpack_int4_np(qzeros)[:, :NG]           # [O, NG]
    zs = z4.astype(np.float32) * scales            # [O, NG]
    jshift = (np.arange(PACK, dtype=np.uint32) * 4)[:, None, None]
    in_maps = []
    KCH = 8
    NKC = NG // KCH

    def chunk_bcast(v):
        # [OS, NG] -> pre-broadcast [NKC, 128, PACK, KCH, NW] bf16
        vj = v.T.reshape(NKC, KCH, PACK, NW).transpose(0, 2, 1, 3)
        return np.ascontiguousarray(
            np.broadcast_to(vj[:, None], (NKC, 128, PACK, KCH, NW))
        ).astype(ml_dtypes.bfloat16)

    for c in range(n_cores):
        sl = slice(c * OS, (c + 1) * OS)
        # repack nibbles o-major: word[i, w] holds o_local = 64*j + w
        t = q4[sl].reshape(PACK, NW, I).astype(np.uint32)   # [j, w, i]
        qwT = np.ascontiguousarray(
            (t << jshift).sum(axis=0, dtype=np.uint32).T).view(np.int32)
        in_maps.append({
            "xT": xT,
            "qw": qwT,
            "sj": chunk_bcast(scales[sl]),
            "zj": chunk_bcast(zs[sl]),
            "bi": np.ascontiguousarray(bias[sl]),
        })
    return in_maps


def kernel(x, qweight, qzeros, scales, bias):
    from concourse.bass_utils import run_bass_kernel_spmd

    B, I = x.shape
    O = qweight.shape[0]
    OS = O // N_CORES
    nc = _get_nc(B, I, OS)
    in_maps = make_in_maps(x, qweight, qzeros, scales, bias)
    res = run_bass_kernel_spmd(nc, in_maps, core_ids=list(range(N_CORES)))
    out = np.concatenate([res.results[c]["out"] for c in range(N_CORES)], axis=1)
    return out.astype(np.float32)
